# revision 1
# baseline (speedup 1.0000x reference)
"""Trainium2 Bass kernel for nn_MultiHeadAttention (B=4, S=2048, D=1024, H=16).

Sharding: 8 cores = batch(4) x head-half(2).  Each core computes, for its
batch element, 8 of the 16 heads: QKV projections against column-sliced
weights, causal attention, and the output projection against the matching
row-slice of Wo.  The two partial outputs per batch element are summed on
the host (replaces the tensor-parallel all-reduce), and Wo_b is added there.

Attention is computed in the transposed-scores layout scoresT[k, q] so the
probs @ V matmul needs no on-chip transposes; the softmax denominator comes
for free from an extra all-ones column appended to V (row 64 of the PV psum
accumulator); normalization runs off the critical path in SBUF.  The kb loop
is software-pipelined one step (scores(kb) issued before pv(kb-1)) so the PE
never sits behind ACT's exp in its in-order queue.
"""

import sys

if "/opt/trn_rl_repo" not in sys.path:
    sys.path.insert(0, "/opt/trn_rl_repo")

import numpy as np
import ml_dtypes

B, S, D = 4, 2048, 1024
H, HD = 16, 64
HH = H // 2          # heads per core
DH = D // 2          # local attention feature dim (HH * HD)
N_CORES = 8
QH = 1024            # q-range processed per attention pass (psum budget)

# matmul dtype mode: "bf16" (fast, ~3e-3 rel err) | "f32" (exact, 4x PE cost)
DT_MODE = "bf16"

_CACHE = {}


def _build(dt_mode):
    import concourse.bass as bass
    import concourse.mybir as mybir
    from concourse import bacc
    from concourse.tile import TileContext
    from concourse.masks import make_upper_triangular

    F32 = mybir.dt.float32
    if dt_mode == "bf16":
        DT = mybir.dt.bfloat16
    elif dt_mode == "f32":
        DT = mybir.dt.float32
    elif dt_mode == "f32r":
        DT = mybir.dt.float32r
    else:
        raise ValueError(dt_mode)

    ADD = mybir.AluOpType.add
    MULT = mybir.AluOpType.mult
    EXP = mybir.ActivationFunctionType.Exp

    nc = bacc.Bacc("TRN2", target_bir_lowering=False, debug=False,
                   num_devices=N_CORES)

    xT = nc.dram_tensor("xT", [D, S], DT, kind="ExternalInput").ap()
    wq = nc.dram_tensor("wq", [D, DH], DT, kind="ExternalInput").ap()
    wk = nc.dram_tensor("wk", [D, DH], DT, kind="ExternalInput").ap()
    wv = nc.dram_tensor("wv", [D, DH], DT, kind="ExternalInput").ap()
    wo = nc.dram_tensor("wo", [DH, D], DT, kind="ExternalInput").ap()
    bq = nc.dram_tensor("bq", [128, DH // 128], F32, kind="ExternalInput").ap()
    bk = nc.dram_tensor("bk", [128, DH // 128], F32, kind="ExternalInput").ap()
    bv = nc.dram_tensor("bv", [128, DH], F32, kind="ExternalInput").ap()
    out = nc.dram_tensor("out", [S, D], F32, kind="ExternalOutput").ap()

    ND = D // 128        # 8 contraction tiles over D
    NS = S // 128        # 16 s-blocks
    NJ = DH // 128       # 4 feature tiles of the local 512 dim
    NSC = S // 512       # 4 columns of 512 over S

    with TileContext(nc) as tc:
        with (
            tc.tile_pool(name="persist", bufs=1) as pp,
            tc.tile_pool(name="qT", bufs=NJ) as pqT,
            tc.tile_pool(name="kT", bufs=NJ) as pkT,
            tc.tile_pool(name="vaug", bufs=NS) as pv,
            tc.tile_pool(name="attnT", bufs=NJ) as pattnT,
        ):
            # ---- constants / biases ----
            bq_t = pp.tile([128, NJ], F32, tag="bq")
            nc.sync.dma_start(bq_t[:], bq[:])
            bk_t = pp.tile([128, NJ], F32, tag="bk")
            nc.sync.dma_start(bk_t[:], bk[:])
            bv_t = pp.tile([128, DH], F32, tag="bv")
            nc.sync.dma_start(bv_t[:], bv[:])
            ones_t = pp.tile([128, HH], F32, tag="ones")
            nc.gpsimd.memset(ones_t[:], 1.0)
            # causal mask for diagonal 128x128 squares of scoresT[k, q]:
            # valid (k <= q) <=> partition p <= free f -> upper-tri incl diag.
            mask_f = pp.tile([128, 128], F32, tag="maskf")
            make_upper_triangular(nc, mask_f[:], val=1.0, diag=True)
            if DT is F32:
                mask_t = mask_f
            else:
                mask_t = pp.tile([128, 128], DT, tag="mask")
                nc.vector.tensor_copy(mask_t[:], mask_f[:])

            # persistent activation buffers
            qT_t = [pqT.tile([128, S], DT, tag="qT", name=f"qT{i}")
                    for i in range(NJ)]
            kT_t = [pkT.tile([128, S], DT, tag="kT", name=f"kT{i}")
                    for i in range(NJ)]
            v_t = [pv.tile([128, HH * (HD + 1)], DT, tag="vaug",
                           name=f"vaug{i}") for i in range(NS)]
            aT_t = [pattnT.tile([128, S], DT, tag="attnT", name=f"attnT{i}")
                    for i in range(NJ)]

            # ================= phase 1: QKV projections =================
            with (
                tc.tile_pool(name="xt", bufs=ND) as pxt,
                tc.tile_pool(name="w", bufs=10) as pw,
                tc.tile_pool(name="qkvps", bufs=4, space="PSUM") as pps,
            ):
                xt_t = [pxt.tile([128, S], DT, tag="xt", name=f"xt{i}")
                        for i in range(ND)]
                for db in range(ND):
                    nc.sync.dma_start(xt_t[db][:], xT[db * 128:(db + 1) * 128, :])

                for name, w_ap, bias_t, dstT in (
                    ("q", wq, bq_t, qT_t), ("k", wk, bk_t, kT_t)
                ):
                    w_t = []
                    for db in range(ND):
                        t = pw.tile([128, DH], DT, tag="w3",
                                    name=f"w{name}{db}")
                        nc.sync.dma_start(t[:], w_ap[db * 128:(db + 1) * 128, :])
                        w_t.append(t)
                    for jb in range(NJ):
                        for sc in range(NSC):
                            ps = pps.tile([128, 512], F32, tag="qkv",
                                          name=f"ps{name}{jb}_{sc}")
                            for db in range(ND):
                                nc.tensor.matmul(
                                    ps[:],
                                    lhsT=w_t[db][:, jb * 128:(jb + 1) * 128],
                                    rhs=xt_t[db][:, sc * 512:(sc + 1) * 512],
                                    start=(db == 0), stop=(db == ND - 1),
                                )
                            nc.vector.tensor_scalar_add(
                                dstT[jb][:, sc * 512:(sc + 1) * 512],
                                ps[:], bias_t[:, jb:jb + 1],
                            )

                # V: normal layout [s, (h, d)] with an appended ones column
                # per head -> v_aug tiles [128, 8*65].
                wv_t = []
                for db in range(ND):
                    t = pw.tile([128, DH], DT, tag="w3", name=f"wv{db}")
                    nc.sync.dma_start(t[:], wv[db * 128:(db + 1) * 128, :])
                    wv_t.append(t)
                for sb in range(NS):
                    ps = pps.tile([128, 512], F32, tag="qkv", name=f"psv{sb}")
                    for db in range(ND):
                        nc.tensor.matmul(
                            ps[:],
                            lhsT=xt_t[db][:, sb * 128:(sb + 1) * 128],
                            rhs=wv_t[db][:],
                            start=(db == 0), stop=(db == ND - 1),
                        )
                    vt = v_t[sb]
                    v3 = vt[:].rearrange("p (h e) -> p h e", e=HD + 1)
                    nc.vector.tensor_tensor(
                        v3[:, :, 0:HD],
                        ps[:].rearrange("p (h e) -> p h e", e=HD),
                        bv_t[:].rearrange("p (h e) -> p h e", e=HD),
                        op=ADD,
                    )
                    nc.vector.tensor_copy(
                        v3[:, :, HD:HD + 1],
                        ones_t[:].rearrange("p (h e) -> p h e", e=1),
                    )

            # ================= phase 2: causal attention =================
            with (
                tc.tile_pool(name="exp", bufs=4) as pexp,
                tc.tile_pool(name="au", bufs=4) as pau,
                tc.tile_pool(name="recip", bufs=3) as prc,
                tc.tile_pool(name="scps", bufs=2, space="PSUM") as pscps,
                tc.tile_pool(name="atps", bufs=2, space="PSUM") as patps,
            ):
                def chunk_cols(lo):
                    chunks = []
                    c = lo
                    while c < QH:
                        c1 = min((c // 512 + 1) * 512, QH)
                        chunks.append((c, c1))
                        c = c1
                    return chunks

                for h in range(HH):
                    hb, hr = h // 2, (h % 2) * 64
                    vcol = h * (HD + 1)
                    for qh in range(S // QH):
                        q0 = qh * QH
                        at = patps.tile([65, QH], F32, tag="at",
                                        name=f"at{h}_{qh}")
                        nkb = (q0 + QH) // 128

                        def scores(kb):
                            k0 = kb * 128
                            lo = max(k0 - q0, 0)
                            sc = pscps.tile([128, QH], F32, tag="sc",
                                            name=f"sc{h}_{qh}_{kb}")
                            for (c0, c1) in chunk_cols(lo):
                                nc.tensor.matmul(
                                    sc[:, c0:c1],
                                    lhsT=kT_t[hb][hr:hr + 64, k0:k0 + 128],
                                    rhs=qT_t[hb][hr:hr + 64, q0 + c0:q0 + c1],
                                    start=True, stop=True,
                                )
                            return sc

                        def exp_pv(kb, sc):
                            k0 = kb * 128
                            lo = max(k0 - q0, 0)
                            et = pexp.tile([128, QH], DT, tag="exp",
                                           name=f"et{h}_{qh}_{kb}")
                            nc.scalar.activation(et[:, lo:QH], sc[:, lo:QH],
                                                 EXP, scale=1.0 / np.sqrt(HD))
                            if k0 >= q0:
                                nc.vector.tensor_mul(et[:, lo:lo + 128],
                                                     et[:, lo:lo + 128],
                                                     mask_t[:])
                            for (c0, c1) in chunk_cols(lo):
                                nc.tensor.matmul(
                                    at[0:65, c0:c1],
                                    lhsT=v_t[kb][:, vcol:vcol + HD + 1],
                                    rhs=et[:, c0:c1],
                                    start=(kb == 0),
                                    stop=(kb == (q0 + c1 - 1) // 128),
                                )

                        # software pipeline: scores one kb ahead of exp+pv so
                        # the in-order PE queue never waits on ACT's exp.
                        prev = scores(0)
                        for kb in range(1, nkb):
                            cur = scores(kb)
                            exp_pv(kb - 1, prev)
                            prev = cur
                        exp_pv(nkb - 1, prev)

                        # Two quick psum->sbuf copies free the attn psum slot;
                        # the normalize runs off the critical path in SBUF.
                        # (reciprocal_approx_fast needs a partition-0 input.)
                        au = pau.tile([64, QH], F32, tag="au",
                                      name=f"au{h}_{qh}")
                        nc.vector.tensor_copy(au[:], at[0:64, :])
                        dn = prc.tile([1, QH], F32, tag="dn", name=f"dn{h}_{qh}")
                        nc.vector.tensor_copy(dn[:], at[64:65, :])
                        rc = prc.tile([1, QH], F32, tag="rc", name=f"rc{h}_{qh}")
                        nc.vector.reciprocal_approx_fast(rc[:], dn[:])
                        bc = prc.tile([64, QH], F32, tag="bc", name=f"bc{h}_{qh}")
                        nc.gpsimd.partition_broadcast(bc[:], rc[:])
                        nc.gpsimd.tensor_tensor(
                            aT_t[hb][hr:hr + 64, q0:q0 + QH],
                            au[:],
                            bc[:],
                            op=MULT,
                        )

            # ================= phase 3: output projection =================
            with (
                tc.tile_pool(name="wo", bufs=NJ) as pwo,
                tc.tile_pool(name="ostage", bufs=4) as post,
                tc.tile_pool(name="ops", bufs=4, space="PSUM") as pops,
            ):
                wo_t = []
                for db in range(NJ):
                    t = pwo.tile([128, D], DT, tag="wo", name=f"wo{db}")
                    nc.sync.dma_start(t[:], wo[db * 128:(db + 1) * 128, :])
                    wo_t.append(t)
                for sb in range(NS):
                    for jc in range(D // 512):
                        ps = pops.tile([128, 512], F32, tag="ops",
                                       name=f"ops{sb}_{jc}")
                        for db in range(NJ):
                            nc.tensor.matmul(
                                ps[:],
                                lhsT=aT_t[db][:, sb * 128:(sb + 1) * 128],
                                rhs=wo_t[db][:, jc * 512:(jc + 1) * 512],
                                start=(db == 0), stop=(db == NJ - 1),
                            )
                        ot = post.tile([128, 512], F32, tag="ostage",
                                       name=f"ot{sb}_{jc}")
                        nc.vector.tensor_copy(ot[:], ps[:])
                        nc.sync.dma_start(
                            out[sb * 128:(sb + 1) * 128, jc * 512:(jc + 1) * 512],
                            ot[:],
                        )

    nc.compile()
    return nc


def _get_nc(dt_mode):
    if dt_mode not in _CACHE:
        _CACHE[dt_mode] = _build(dt_mode)
    return _CACHE[dt_mode]


def make_in_maps(x, Wq_w, Wq_b, Wk_w, Wk_b, Wv_w, Wv_b, Wo_w, Wo_b, np_dt):
    in_maps = []
    for core in range(N_CORES):
        b, half = core // 2, core % 2
        sl = slice(half * DH, (half + 1) * DH)
        in_maps.append({
            "xT": np.ascontiguousarray(x[b].T).astype(np_dt),
            "wq": np.ascontiguousarray(Wq_w[:, sl]).astype(np_dt),
            "wk": np.ascontiguousarray(Wk_w[:, sl]).astype(np_dt),
            "wv": np.ascontiguousarray(Wv_w[:, sl]).astype(np_dt),
            "wo": np.ascontiguousarray(Wo_w[sl, :]).astype(np_dt),
            "bq": np.ascontiguousarray(Wq_b[sl].reshape(-1, 128).T),
            "bk": np.ascontiguousarray(Wk_b[sl].reshape(-1, 128).T),
            "bv": np.broadcast_to(Wv_b[sl], (128, DH)).copy(),
        })
    return in_maps


def kernel(x, Wq_w, Wq_b, Wk_w, Wk_b, Wv_w, Wv_b, Wo_w, Wo_b):
    from concourse.bass_utils import run_bass_kernel_spmd

    np_dt = ml_dtypes.bfloat16 if DT_MODE == "bf16" else np.float32

    args = [np.asarray(a, np.float32) for a in
            (x, Wq_w, Wq_b, Wk_w, Wk_b, Wv_w, Wv_b, Wo_w, Wo_b)]
    x, Wq_w, Wq_b, Wk_w, Wk_b, Wv_w, Wv_b, Wo_w, Wo_b = args

    nc = _get_nc(DT_MODE)
    in_maps = make_in_maps(x, Wq_w, Wq_b, Wk_w, Wk_b, Wv_w, Wv_b, Wo_w, Wo_b,
                           np_dt)
    res = run_bass_kernel_spmd(nc, in_maps, list(range(N_CORES)))

    out = np.empty((B, S, D), np.float32)
    for b in range(B):
        out[b] = res.results[2 * b]["out"] + res.results[2 * b + 1]["out"] + Wo_b
    return out



# revision 5
# speedup vs baseline: 1.0572x; 1.0572x over previous
"""Trainium2 Bass kernel for nn_MultiHeadAttention (B=4, S=2048, D=1024, H=16).

Sharding: 8 cores = batch(4) x head-half(2).  Each core computes, for its
batch element, 8 of the 16 heads: QKV projections against column-sliced
weights, causal attention, and the output projection against the matching
row-slice of Wo.  The two partial outputs per batch element are summed on
the host (replaces the tensor-parallel all-reduce), and Wo_b is added there.

Attention runs in the transposed-scores layout scoresT[k, q]; the softmax
denominator comes free from an all-ones column appended to V (row 64 of the
PV psum accumulator).  Heads are processed in PAIRS sharing one [128, 1024]
scores psum tile (head A in cols 0:512, head B in 512:1024) so one ACTIVATE
exps both heads' scores; q is chunked at 512.

The whole kernel is software-pipelined around the ACT engine's exp stream
(the irreducible ~120us of work): Q/K projections for the NEXT head pair
and output-projection tiles for finished pairs are injected as fill between
attention steps so the PE never idles (idle gaps also drop the PE's DVFS
p-state from 2.4 to 1.2 GHz).  PSUM: 2 scores bufs (4 banks) + 1 shared PV
accumulator (2 banks) + 2 fill bufs (2 banks).
"""

import sys

if "/opt/trn_rl_repo" not in sys.path:
    sys.path.insert(0, "/opt/trn_rl_repo")

import numpy as np
import ml_dtypes

B, S, D = 4, 2048, 1024
H, HD = 16, 64
HH = H // 2          # heads per core
DH = D // 2          # local attention feature dim (HH * HD)
N_CORES = 8
QC = 512             # q-chunk per attention pass (1 psum bank per head)

# matmul dtype mode: "bf16" (fast, ~3e-3 rel err) | "f32" (exact, 4x PE cost)
DT_MODE = "bf16"

_CACHE = {}


def _build(dt_mode):
    import concourse.bass as bass
    import concourse.mybir as mybir
    from concourse import bacc
    from concourse.tile import TileContext
    from concourse.masks import make_upper_triangular

    F32 = mybir.dt.float32
    if dt_mode == "bf16":
        DT = mybir.dt.bfloat16
    elif dt_mode == "f32":
        DT = mybir.dt.float32
    else:
        raise ValueError(dt_mode)

    ADD = mybir.AluOpType.add
    MULT = mybir.AluOpType.mult
    EXP = mybir.ActivationFunctionType.Exp

    nc = bacc.Bacc("TRN2", target_bir_lowering=False, debug=False,
                   num_devices=N_CORES)

    xT = nc.dram_tensor("xT", [D, S], DT, kind="ExternalInput").ap()
    wq = nc.dram_tensor("wq", [D, DH], DT, kind="ExternalInput").ap()
    wk = nc.dram_tensor("wk", [D, DH], DT, kind="ExternalInput").ap()
    wv = nc.dram_tensor("wv", [D, DH], DT, kind="ExternalInput").ap()
    wo = nc.dram_tensor("wo", [DH, D], DT, kind="ExternalInput").ap()
    bq = nc.dram_tensor("bq", [128, DH // 128], F32, kind="ExternalInput").ap()
    bk = nc.dram_tensor("bk", [128, DH // 128], F32, kind="ExternalInput").ap()
    bv = nc.dram_tensor("bv", [128, DH], F32, kind="ExternalInput").ap()
    out = nc.dram_tensor("out", [S, D], F32, kind="ExternalOutput").ap()

    ND = D // 128        # 8 contraction tiles over D
    NS = S // 128        # 16 s-blocks
    NJ = DH // 128       # 4 head-pair tiles of the local 512 dim
    NSC = S // 512       # 4 columns of 512 over S
    NP = S // QC         # 4 q-chunk passes

    with TileContext(nc) as tc:
        with (
            tc.tile_pool(name="persist", bufs=1) as pp,
            tc.tile_pool(name="xt", bufs=ND) as pxt,
            tc.tile_pool(name="wqk", bufs=2 * ND) as pwqk,
            tc.tile_pool(name="wv", bufs=ND) as pwv,
            tc.tile_pool(name="wo", bufs=NJ) as pwo,
            tc.tile_pool(name="qT", bufs=NJ) as pqT,
            tc.tile_pool(name="kT", bufs=NJ) as pkT,
            tc.tile_pool(name="vaug", bufs=NS) as pv,
            tc.tile_pool(name="attnT", bufs=NJ) as pattnT,
            tc.tile_pool(name="exp", bufs=3) as pexp,
            tc.tile_pool(name="au", bufs=2) as pau,
            tc.tile_pool(name="dn", bufs=4) as pdn,
            tc.tile_pool(name="bc", bufs=2) as pbc,
            tc.tile_pool(name="ostage", bufs=4) as post,
            tc.tile_pool(name="scps", bufs=2, space="PSUM") as pscps,
            tc.tile_pool(name="atps", bufs=1, space="PSUM") as patps,
            tc.tile_pool(name="auxps", bufs=2, space="PSUM") as pauxps,
        ):
            # ---- input DMAs (ordered by first use) ----
            wq_t, wk_t, xt_t = [], [], []
            for db in range(ND):
                tq = pwqk.tile([128, DH], DT, tag="wqk", name=f"wq{db}")
                nc.sync.dma_start(tq[:], wq[db * 128:(db + 1) * 128, :])
                wq_t.append(tq)
                tx = pxt.tile([128, S], DT, tag="xt", name=f"xt{db}")
                nc.sync.dma_start(tx[:], xT[db * 128:(db + 1) * 128, :])
                xt_t.append(tx)
            for db in range(ND):
                tk = pwqk.tile([128, DH], DT, tag="wqk", name=f"wk{db}")
                nc.sync.dma_start(tk[:], wk[db * 128:(db + 1) * 128, :])
                wk_t.append(tk)
            wv_t = []
            for db in range(ND):
                t = pwv.tile([128, DH], DT, tag="wv", name=f"wv{db}")
                nc.sync.dma_start(t[:], wv[db * 128:(db + 1) * 128, :])
                wv_t.append(t)
            bq_t = pp.tile([128, NJ], F32, tag="bq")
            nc.sync.dma_start(bq_t[:], bq[:])
            bk_t = pp.tile([128, NJ], F32, tag="bk")
            nc.sync.dma_start(bk_t[:], bk[:])
            bv_t = pp.tile([128, DH], F32, tag="bv")
            nc.sync.dma_start(bv_t[:], bv[:])
            wo_t = []
            for db in range(NJ):
                t = pwo.tile([128, D], DT, tag="wo", name=f"wo{db}")
                nc.sync.dma_start(t[:], wo[db * 128:(db + 1) * 128, :])
                wo_t.append(t)

            # ---- constants ----
            ones_t = pp.tile([128, HH], F32, tag="ones")
            nc.gpsimd.memset(ones_t[:], 1.0)
            # causal mask for diagonal 128x128 squares of scoresT[k, q]:
            # valid (k <= q) <=> partition p <= free f -> upper-tri incl
            # diag; two side-by-side copies (one per head of a pair).
            mask_f = pp.tile([128, 128], F32, tag="maskf")
            make_upper_triangular(nc, mask_f[:], val=1.0, diag=True)
            mask2 = pp.tile([128, 256], DT, tag="mask2")
            nc.vector.tensor_copy(mask2[:, 0:128], mask_f[:])
            nc.vector.tensor_copy(mask2[:, 128:256], mask_f[:])
            mask23 = mask2[:].rearrange("p (h c) -> p h c", h=2)

            # persistent activation buffers
            qT_t = [pqT.tile([128, S], DT, tag="qT", name=f"qT{i}")
                    for i in range(NJ)]
            kT_t = [pkT.tile([128, S], DT, tag="kT", name=f"kT{i}")
                    for i in range(NJ)]
            v_t = [pv.tile([128, HH * (HD + 1)], DT, tag="vaug",
                           name=f"vaug{i}") for i in range(NS)]
            aT_t = [pattnT.tile([128, S], DT, tag="attnT", name=f"attnT{i}")
                    for i in range(NJ)]

            # ---------- fill-work generators (2 matmuls per piece) ----------
            def qk_pieces(j):
                """Q/K projection for head pair j, as ~0.4us PE pieces."""
                pieces = []
                for nm, w_t, bias_t, dstT in (
                    ("q", wq_t, bq_t, qT_t), ("k", wk_t, bk_t, kT_t)
                ):
                    for sc in range(NSC):
                        box = {}
                        for db0 in range(0, ND, 2):
                            def piece(db0=db0, nm=nm, w_t=w_t, bias_t=bias_t,
                                      dstT=dstT, sc=sc, j=j, box=box):
                                if db0 == 0:
                                    box["t"] = pauxps.tile(
                                        [128, 512], F32, tag="aux",
                                        name=f"qk{nm}{j}_{sc}")
                                for db in (db0, db0 + 1):
                                    nc.tensor.matmul(
                                        box["t"][:],
                                        lhsT=w_t[db][:, j * 128:(j + 1) * 128],
                                        rhs=xt_t[db][:, sc * 512:(sc + 1) * 512],
                                        start=(db == 0), stop=(db == ND - 1),
                                    )
                                if db0 == ND - 2:
                                    nc.vector.tensor_scalar_add(
                                        dstT[j][:, sc * 512:(sc + 1) * 512],
                                        box["t"][:], bias_t[:, j:j + 1],
                                    )
                            pieces.append(piece)
                return pieces

            def v_pieces(sb):
                """V projection for s-block sb (all 8 heads + ones col)."""
                pieces = []
                box = {}
                for db0 in range(0, ND, 2):
                    def piece(db0=db0, sb=sb, box=box):
                        if db0 == 0:
                            box["t"] = pauxps.tile([128, 512], F32, tag="aux",
                                                   name=f"vps{sb}")
                        for db in (db0, db0 + 1):
                            nc.tensor.matmul(
                                box["t"][:],
                                lhsT=xt_t[db][:, sb * 128:(sb + 1) * 128],
                                rhs=wv_t[db][:],
                                start=(db == 0), stop=(db == ND - 1),
                            )
                        if db0 == ND - 2:
                            vt = v_t[sb]
                            v3 = vt[:].rearrange("p (h e) -> p h e", e=HD + 1)
                            nc.vector.tensor_tensor(
                                v3[:, :, 0:HD],
                                box["t"][:].rearrange("p (h e) -> p h e", e=HD),
                                bv_t[:].rearrange("p (h e) -> p h e", e=HD),
                                op=ADD,
                            )
                            nc.vector.tensor_copy(
                                v3[:, :, HD:HD + 1],
                                ones_t[:].rearrange("p (h e) -> p h e", e=1),
                            )
                    pieces.append(piece)
                return pieces

            def outproj_pieces(sb):
                """Output projection for s-block sb: 2 jc chunks x 2 pieces."""
                pieces = []
                for jc in range(D // 512):
                    box = {}
                    for db0 in range(0, NJ, 2):
                        def piece(db0=db0, sb=sb, jc=jc, box=box):
                            if db0 == 0:
                                box["t"] = pauxps.tile(
                                    [128, 512], F32, tag="aux",
                                    name=f"ops{sb}_{jc}")
                            for db in (db0, db0 + 1):
                                nc.tensor.matmul(
                                    box["t"][:],
                                    lhsT=aT_t[db][:, sb * 128:(sb + 1) * 128],
                                    rhs=wo_t[db][:, jc * 512:(jc + 1) * 512],
                                    start=(db == 0), stop=(db == NJ - 1),
                                )
                            if db0 == NJ - 2:
                                ot = post.tile([128, 512], F32, tag="ostage",
                                               name=f"ot{sb}_{jc}")
                                nc.vector.tensor_copy(ot[:], box["t"][:])
                                nc.sync.dma_start(
                                    out[sb * 128:(sb + 1) * 128,
                                        jc * 512:(jc + 1) * 512],
                                    ot[:],
                                )
                        pieces.append(piece)
                return pieces

            # ---------- attention for one head pair, with fill ----------
            def attention_pair(j, fill):
                """fill: list of per-pass piece-lists (len NP)."""
                vcA = (2 * j) * (HD + 1)
                vcB = (2 * j + 1) * (HD + 1)
                for p in range(NP):
                    q0 = p * QC
                    nkb = (q0 + QC) // 128
                    at2 = patps.tile([65, 2 * QC], F32, tag="at",
                                     name=f"at{j}_{p}")
                    pfill = fill[p]
                    fi = 0

                    def scores(kb):
                        k0 = kb * 128
                        lo = max(k0 - q0, 0)
                        sc2 = pscps.tile([128, 2 * QC], F32, tag="sc",
                                         name=f"sc{j}_{p}_{kb}")
                        for hi, hr in ((0, 0), (1, 64)):
                            nc.tensor.matmul(
                                sc2[:, hi * QC + lo:(hi + 1) * QC],
                                lhsT=kT_t[j][hr:hr + 64, k0:k0 + 128],
                                rhs=qT_t[j][hr:hr + 64, q0 + lo:q0 + QC],
                                start=True, stop=True,
                            )
                        return sc2

                    def exp_pv(kb, sc2):
                        k0 = kb * 128
                        lo = max(k0 - q0, 0)
                        et = pexp.tile([128, 2 * QC], DT, tag="exp",
                                       name=f"et{j}_{p}_{kb}")
                        et3 = et[:].rearrange("p (h c) -> p h c", h=2)
                        sc3 = sc2[:].rearrange("p (h c) -> p h c", h=2)
                        nc.scalar.activation(
                            et3[:, :, lo:QC], sc3[:, :, lo:QC],
                            EXP, scale=1.0 / np.sqrt(HD),
                        )
                        if k0 >= q0:
                            nc.vector.tensor_tensor(
                                et3[:, :, lo:lo + 128],
                                et3[:, :, lo:lo + 128],
                                mask23, op=MULT,
                            )
                        for hi, vc in ((0, vcA), (1, vcB)):
                            nc.tensor.matmul(
                                at2[0:65, hi * QC + lo:(hi + 1) * QC],
                                lhsT=v_t[kb][:, vc:vc + HD + 1],
                                rhs=et[:, hi * QC + lo:(hi + 1) * QC],
                                start=(kb == 0), stop=(kb == nkb - 1),
                            )

                    pend = {}
                    for kb in range(min(2, nkb)):
                        pend[kb] = scores(kb)
                    for kb in range(nkb):
                        want = ((kb + 1) * len(pfill)) // nkb
                        while fi < want:
                            pfill[fi]()
                            fi += 1
                        # exp_pv(kb) BEFORE scores(kb+2): the scps pool has
                        # 2 bufs, so scores(kb+2) reuses sc2(kb)'s buffer
                        # and its WAR dep must see exp(kb) already issued.
                        exp_pv(kb, pend.pop(kb))
                        if kb + 2 < nkb:
                            pend[kb + 2] = scores(kb + 2)
                    while fi < len(pfill):
                        pfill[fi]()
                        fi += 1

                    # one DVE copy frees the at2 psum slot; the normalize
                    # runs off the critical path (baseline-proven chain).
                    au = pau.tile([65, 2 * QC], F32, tag="au",
                                  name=f"au{j}_{p}")
                    nc.vector.tensor_copy(au[:], at2[0:65, :])
                    dn = pdn.tile([1, 2 * QC], F32, tag="dn", name=f"dn{j}_{p}")
                    nc.vector.tensor_copy(dn[:], au[64:65, :])
                    rc = pdn.tile([1, 2 * QC], F32, tag="rc", name=f"rc{j}_{p}")
                    nc.vector.reciprocal_approx_fast(rc[:], dn[:])
                    bcb = pbc.tile([64, 2 * QC], F32, tag="bc",
                                   name=f"bc{j}_{p}")
                    nc.gpsimd.partition_broadcast(bcb[:], rc[:])
                    for hi, hr in ((0, 0), (1, 64)):
                        nc.gpsimd.tensor_tensor(
                            aT_t[j][hr:hr + 64, q0:q0 + QC],
                            au[0:64, hi * QC:(hi + 1) * QC],
                            bcb[:, hi * QC:(hi + 1) * QC],
                            op=MULT,
                        )

            # ---------------- schedule ----------------
            # prologue: QK(0) + all V (PE warm-up while DMAs stream)
            for piece in qk_pieces(0):
                piece()
            for sb in range(NS):
                for piece in v_pieces(sb):
                    piece()

            # windows 0..2: attention(j) + QK(j+1) as fill
            for j in range(NJ - 1):
                qk = qk_pieces(j + 1)
                per = (len(qk) + NP - 1) // NP
                fill = [qk[p * per:(p + 1) * per] for p in range(NP)]
                attention_pair(j, fill)

            # window 3: attention(3) + out-proj of s-blocks gated on the
            # pass that produced their aT columns (pass p covers q-chunk p,
            # so sb 4(p-1)..4p-1 are ready when pass p starts).
            fill = [[] for _ in range(NP)]
            for p in range(1, NP):
                for sb in range(4 * (p - 1), 4 * p):
                    fill[p].extend(outproj_pieces(sb))
            attention_pair(NJ - 1, fill)

            # epilogue: remaining out-proj
            for sb in range(4 * (NP - 1), NS):
                for piece in outproj_pieces(sb):
                    piece()

    nc.compile()
    return nc


def _get_nc(dt_mode):
    if dt_mode not in _CACHE:
        _CACHE[dt_mode] = _build(dt_mode)
    return _CACHE[dt_mode]


def make_in_maps(x, Wq_w, Wq_b, Wk_w, Wk_b, Wv_w, Wv_b, Wo_w, Wo_b, np_dt):
    in_maps = []
    for core in range(N_CORES):
        b, half = core // 2, core % 2
        sl = slice(half * DH, (half + 1) * DH)
        in_maps.append({
            "xT": np.ascontiguousarray(x[b].T).astype(np_dt),
            "wq": np.ascontiguousarray(Wq_w[:, sl]).astype(np_dt),
            "wk": np.ascontiguousarray(Wk_w[:, sl]).astype(np_dt),
            "wv": np.ascontiguousarray(Wv_w[:, sl]).astype(np_dt),
            "wo": np.ascontiguousarray(Wo_w[sl, :]).astype(np_dt),
            "bq": np.ascontiguousarray(Wq_b[sl].reshape(-1, 128).T),
            "bk": np.ascontiguousarray(Wk_b[sl].reshape(-1, 128).T),
            "bv": np.broadcast_to(Wv_b[sl], (128, DH)).copy(),
        })
    return in_maps


def kernel(x, Wq_w, Wq_b, Wk_w, Wk_b, Wv_w, Wv_b, Wo_w, Wo_b):
    from concourse.bass_utils import run_bass_kernel_spmd

    np_dt = ml_dtypes.bfloat16 if DT_MODE == "bf16" else np.float32

    args = [np.asarray(a, np.float32) for a in
            (x, Wq_w, Wq_b, Wk_w, Wk_b, Wv_w, Wv_b, Wo_w, Wo_b)]
    x, Wq_w, Wq_b, Wk_w, Wk_b, Wv_w, Wv_b, Wo_w, Wo_b = args

    nc = _get_nc(DT_MODE)
    in_maps = make_in_maps(x, Wq_w, Wq_b, Wk_w, Wk_b, Wv_w, Wv_b, Wo_w, Wo_b,
                           np_dt)
    res = run_bass_kernel_spmd(nc, in_maps, list(range(N_CORES)))

    out = np.empty((B, S, D), np.float32)
    for b in range(B):
        out[b] = res.results[2 * b]["out"] + res.results[2 * b + 1]["out"] + Wo_b
    return out


# revision 19
# speedup vs baseline: 1.1321x; 1.0708x over previous
"""Trainium2 Bass kernel for nn_MultiHeadAttention (B=4, S=2048, D=1024, H=16).

Sharding: 8 cores = batch(4) x head-half(2).  Each core computes, for its
batch element, 8 of the 16 heads: QKV projections against column-sliced
weights, causal attention, and the output projection against the matching
row-slice of Wo.  The two partial outputs per batch element are summed on
the host (replaces the tensor-parallel all-reduce), and Wo_b is added there.

Attention runs in the transposed-scores layout scoresT[k, q]; the softmax
denominator comes free from an all-ones column appended to V (row 64 of the
PV psum accumulator).  Heads are processed in PAIRS sharing one [128, 1024]
scores psum tile (head A in cols 0:512, head B in 512:1024) so one ACTIVATE
exps both heads' scores; q is chunked at 512.

The whole kernel is software-pipelined around the ACT engine's exp stream
(the irreducible ~120us of work): Q/K projections for the NEXT head pair
and output-projection tiles for finished pairs are injected as fill between
attention steps so the PE never idles (idle gaps also drop the PE's DVFS
p-state from 2.4 to 1.2 GHz).  PSUM: 2 scores bufs (4 banks) + 1 shared PV
accumulator (2 banks) + 2 fill bufs (2 banks).
"""

import sys

if "/opt/trn_rl_repo" not in sys.path:
    sys.path.insert(0, "/opt/trn_rl_repo")

import numpy as np
import ml_dtypes

B, S, D = 4, 2048, 1024
H, HD = 16, 64
HH = H // 2          # heads per core
DH = D // 2          # local attention feature dim (HH * HD)
N_CORES = 8
QC = 512             # q-chunk per attention pass (1 psum bank per head)

# matmul dtype mode: "bf16" (fast, ~3e-3 rel err) | "f32" (exact, 4x PE cost)
DT_MODE = "bf16"

_CACHE = {}


def _build(dt_mode):
    import concourse.bass as bass
    import concourse.mybir as mybir
    from concourse import bacc
    from concourse.tile import TileContext
    from concourse.masks import make_upper_triangular

    F32 = mybir.dt.float32
    if dt_mode == "bf16":
        DT = mybir.dt.bfloat16
    elif dt_mode == "f32":
        DT = mybir.dt.float32
    else:
        raise ValueError(dt_mode)
    FP8 = mybir.dt.float8e4
    DR = mybir.MatmulPerfMode.DoubleRow

    ADD = mybir.AluOpType.add
    MULT = mybir.AluOpType.mult
    EXP = mybir.ActivationFunctionType.Exp

    nc = bacc.Bacc("TRN2", target_bir_lowering=False, debug=False,
                   num_devices=N_CORES)

    xT = nc.dram_tensor("xT", [D, S], DT, kind="ExternalInput").ap()
    wq = nc.dram_tensor("wq", [D, DH], DT, kind="ExternalInput").ap()
    wk = nc.dram_tensor("wk", [D, DH], DT, kind="ExternalInput").ap()
    wv = nc.dram_tensor("wv", [D, DH], DT, kind="ExternalInput").ap()
    wo = nc.dram_tensor("wo", [DH, D], DT, kind="ExternalInput").ap()
    bq = nc.dram_tensor("bq", [128, DH // 128], F32, kind="ExternalInput").ap()
    bk = nc.dram_tensor("bk", [128, DH // 128], F32, kind="ExternalInput").ap()
    bv = nc.dram_tensor("bv", [128, DH], F32, kind="ExternalInput").ap()
    out = nc.dram_tensor("out", [S, D], F32, kind="ExternalOutput").ap()

    ND = D // 128        # 8 contraction tiles over D
    NS = S // 128        # 16 s-blocks
    NJ = DH // 128       # 4 head-pair tiles of the local 512 dim
    NSC = S // 512       # 4 columns of 512 over S
    NP = S // QC         # 4 q-chunk passes

    with TileContext(nc) as tc:
        with (
            tc.tile_pool(name="persist", bufs=1) as pp,
            tc.tile_pool(name="xt", bufs=ND) as pxt,
            tc.tile_pool(name="wqk", bufs=2 * ND) as pwqk,
            tc.tile_pool(name="wv", bufs=ND) as pwv,
            tc.tile_pool(name="wo", bufs=NJ) as pwo,
            tc.tile_pool(name="qT", bufs=NJ) as pqT,
            tc.tile_pool(name="kT", bufs=NJ) as pkT,
            tc.tile_pool(name="vaug", bufs=NS) as pv,
            tc.tile_pool(name="attnT", bufs=NJ) as pattnT,
            tc.tile_pool(name="exp", bufs=3) as pexp,
            tc.tile_pool(name="au", bufs=2) as pau,
            tc.tile_pool(name="dn", bufs=4) as pdn,
            tc.tile_pool(name="bc", bufs=2) as pbc,
            tc.tile_pool(name="ostage", bufs=4) as post,
            tc.tile_pool(name="scps", bufs=2, space="PSUM") as pscps,
            tc.tile_pool(name="atps", bufs=1, space="PSUM") as patps,
            tc.tile_pool(name="auxps", bufs=2, space="PSUM") as pauxps,
        ):
            # ---- input DMAs (ordered by first use) ----
            wq_t, wk_t, xt_t = [], [], []
            for db in range(ND):
                tq = pwqk.tile([128, DH], DT, tag="wqk", name=f"wq{db}")
                nc.sync.dma_start(tq[:], wq[db * 128:(db + 1) * 128, :])
                wq_t.append(tq)
                tx = pxt.tile([128, S], DT, tag="xt", name=f"xt{db}")
                nc.sync.dma_start(tx[:], xT[db * 128:(db + 1) * 128, :])
                xt_t.append(tx)
            for db in range(ND):
                tk = pwqk.tile([128, DH], DT, tag="wqk", name=f"wk{db}")
                nc.sync.dma_start(tk[:], wk[db * 128:(db + 1) * 128, :])
                wk_t.append(tk)
            wv_t = []
            for db in range(ND):
                t = pwv.tile([128, DH], DT, tag="wv", name=f"wv{db}")
                nc.sync.dma_start(t[:], wv[db * 128:(db + 1) * 128, :])
                wv_t.append(t)
            bq_t = pp.tile([128, NJ], F32, tag="bq")
            nc.sync.dma_start(bq_t[:], bq[:])
            bk_t = pp.tile([128, NJ], F32, tag="bk")
            nc.sync.dma_start(bk_t[:], bk[:])
            bv_t = pp.tile([128, DH], F32, tag="bv")
            nc.sync.dma_start(bv_t[:], bv[:])
            wo_t = []
            for db in range(NJ):
                t = pwo.tile([128, D], DT, tag="wo", name=f"wo{db}")
                nc.sync.dma_start(t[:], wo[db * 128:(db + 1) * 128, :])
                wo_t.append(t)

            # ---- constants ----
            ones_t = pp.tile([128, HH], F32, tag="ones")
            nc.gpsimd.memset(ones_t[:], 1.0)
            neg2_t = pp.tile([128, 1], F32, tag="neg2")
            nc.gpsimd.memset(neg2_t[:], -2.0)
            # causal mask for diagonal 128x128 squares of scoresT[k, q]:
            # valid (k <= q) <=> partition p <= free f -> upper-tri incl
            # diag; two side-by-side copies (one per head of a pair).
            mask_f = pp.tile([128, 128], F32, tag="maskf")
            make_upper_triangular(nc, mask_f[:], val=1.0, diag=True)
            mask2 = pp.tile([128, 256], DT, tag="mask2")
            nc.vector.tensor_copy(mask2[:, 0:128], mask_f[:])
            nc.vector.tensor_copy(mask2[:, 128:256], mask_f[:])
            mask23 = mask2[:].rearrange("p (h c) -> p h c", h=2)

            # persistent activation buffers
            qT_t = [pqT.tile([128, S], DT, tag="qT", name=f"qT{i}")
                    for i in range(NJ)]
            kT_t = [pkT.tile([128, S], DT, tag="kT", name=f"kT{i}")
                    for i in range(NJ)]
            v_t = [pv.tile([128, HH * (HD + 1)], DT, tag="vaug",
                           name=f"vaug{i}") for i in range(NS)]
            aT_t = [pattnT.tile([128, S], DT, tag="attnT", name=f"attnT{i}")
                    for i in range(NJ)]

            # ---------- fill-work generators (2 matmuls per piece) ----------
            def qk_pieces(j):
                """Q/K projection for head pair j, as ~0.4us PE pieces."""
                pieces = []
                for nm, w_t, bias_t, dstT in (
                    ("q", wq_t, bq_t, qT_t), ("k", wk_t, bk_t, kT_t)
                ):
                    for sc in range(NSC):
                        box = {}
                        for db0 in range(0, ND, 2):
                            def piece(db0=db0, nm=nm, w_t=w_t, bias_t=bias_t,
                                      dstT=dstT, sc=sc, j=j, box=box):
                                if db0 == 0:
                                    box["t"] = pauxps.tile(
                                        [128, 512], F32, tag="aux",
                                        name=f"qk{nm}{j}_{sc}")
                                for db in (db0, db0 + 1):
                                    nc.tensor.matmul(
                                        box["t"][:],
                                        lhsT=w_t[db][:, j * 128:(j + 1) * 128],
                                        rhs=xt_t[db][:, sc * 512:(sc + 1) * 512],
                                        start=(db == 0), stop=(db == ND - 1),
                                    )
                                if db0 == ND - 2:
                                    nc.vector.tensor_scalar_add(
                                        dstT[j][:, sc * 512:(sc + 1) * 512],
                                        box["t"][:], bias_t[:, j:j + 1],
                                    )
                            pieces.append(piece)
                return pieces

            def v_pieces(sb):
                """V projection for s-block sb (all 8 heads + ones col)."""
                pieces = []
                box = {}
                for db0 in range(0, ND, 2):
                    def piece(db0=db0, sb=sb, box=box):
                        if db0 == 0:
                            box["t"] = pauxps.tile([128, 512], F32, tag="aux",
                                                   name=f"vps{sb}")
                        for db in (db0, db0 + 1):
                            nc.tensor.matmul(
                                box["t"][:],
                                lhsT=xt_t[db][:, sb * 128:(sb + 1) * 128],
                                rhs=wv_t[db][:],
                                start=(db == 0), stop=(db == ND - 1),
                            )
                        if db0 == ND - 2:
                            vt = v_t[sb]
                            v3 = vt[:].rearrange("p (h e) -> p h e", e=HD + 1)
                            nc.vector.tensor_tensor(
                                v3[:, :, 0:HD],
                                box["t"][:].rearrange("p (h e) -> p h e", e=HD),
                                bv_t[:].rearrange("p (h e) -> p h e", e=HD),
                                op=ADD,
                            )
                            nc.vector.tensor_copy(
                                v3[:, :, HD:HD + 1],
                                ones_t[:].rearrange("p (h e) -> p h e", e=1),
                            )
                    pieces.append(piece)
                return pieces

            def outproj_pieces(sb):
                """Output projection for s-block sb: 2 jc chunks x 2 pieces."""
                pieces = []
                for jc in range(D // 512):
                    box = {}
                    for db0 in range(0, NJ, 2):
                        def piece(db0=db0, sb=sb, jc=jc, box=box):
                            if db0 == 0:
                                box["t"] = pauxps.tile(
                                    [128, 512], F32, tag="aux",
                                    name=f"ops{sb}_{jc}")
                            for db in (db0, db0 + 1):
                                nc.tensor.matmul(
                                    box["t"][:],
                                    lhsT=aT_t[db][:, sb * 128:(sb + 1) * 128],
                                    rhs=wo_t[db][:, jc * 512:(jc + 1) * 512],
                                    start=(db == 0), stop=(db == NJ - 1),
                                )
                            if db0 == NJ - 2:
                                ot = post.tile([128, 512], F32, tag="ostage",
                                               name=f"ot{sb}_{jc}")
                                nc.vector.tensor_copy(ot[:], box["t"][:])
                                nc.sync.dma_start(
                                    out[sb * 128:(sb + 1) * 128,
                                        jc * 512:(jc + 1) * 512],
                                    ot[:],
                                )
                        pieces.append(piece)
                return pieces

            # ---------- attention for one head pair, with fill ----------
            def attention_pair(j, fill):
                """fill: list of per-pass piece-lists (len NP)."""
                vcA = (2 * j) * (HD + 1)
                vcB = (2 * j + 1) * (HD + 1)
                for p in range(NP):
                    q0 = p * QC
                    nkb = (q0 + QC) // 128
                    at2 = patps.tile([65, 2 * QC], F32, tag="at",
                                     name=f"at{j}_{p}")
                    pfill = fill[p]
                    fi = 0

                    def scores(kb):
                        k0 = kb * 128
                        lo = max(k0 - q0, 0)
                        sc2 = pscps.tile([128, 2 * QC], F32, tag="sc",
                                         name=f"sc{j}_{p}_{kb}")
                        for hi, hr in ((0, 0), (1, 64)):
                            nc.tensor.matmul(
                                sc2[:, hi * QC + lo:(hi + 1) * QC],
                                lhsT=kT_t[j][hr:hr + 64, k0:k0 + 128],
                                rhs=qT_t[j][hr:hr + 64, q0 + lo:q0 + QC],
                                start=True, stop=True,
                            )
                        return sc2

                    def exp_pv(kb, sc2):
                        k0 = kb * 128
                        lo = max(k0 - q0, 0)
                        et = pexp.tile([128, 2 * QC], DT, tag="exp",
                                       name=f"et{j}_{p}_{kb}")
                        et3 = et[:].rearrange("p (h c) -> p h c", h=2)
                        sc3 = sc2[:].rearrange("p (h c) -> p h c", h=2)
                        nc.scalar.activation(
                            et3[:, :, lo:QC], sc3[:, :, lo:QC],
                            EXP, scale=1.0 / np.sqrt(HD),
                        )
                        if k0 >= q0:
                            nc.vector.tensor_tensor(
                                et3[:, :, lo:lo + 128],
                                et3[:, :, lo:lo + 128],
                                mask23, op=MULT,
                            )
                        for hi, vc in ((0, vcA), (1, vcB)):
                            nc.tensor.matmul(
                                at2[0:65, hi * QC + lo:(hi + 1) * QC],
                                lhsT=v_t[kb][:, vc:vc + HD + 1],
                                rhs=et[:, hi * QC + lo:(hi + 1) * QC],
                                start=(kb == 0), stop=(kb == nkb - 1),
                            )

                    pend = {}
                    for kb in range(min(2, nkb)):
                        pend[kb] = scores(kb)
                    for kb in range(nkb):
                        want = ((kb + 1) * len(pfill)) // nkb
                        while fi < want:
                            pfill[fi]()
                            fi += 1
                        # exp_pv(kb) BEFORE scores(kb+2): the scps pool has
                        # 2 bufs, so scores(kb+2) reuses sc2(kb)'s buffer
                        # and its WAR dep must see exp(kb) already issued.
                        exp_pv(kb, pend.pop(kb))
                        if kb + 2 < nkb:
                            pend[kb + 2] = scores(kb + 2)
                    while fi < len(pfill):
                        pfill[fi]()
                        fi += 1

                    # one DVE copy frees the at2 psum slot; the normalize
                    # runs off the critical path (baseline-proven chain).
                    au = pau.tile([65, 2 * QC], F32, tag="au",
                                  name=f"au{j}_{p}")
                    nc.vector.tensor_copy(au[:], at2[0:65, :])
                    dn = pdn.tile([1, 2 * QC], F32, tag="dn", name=f"dn{j}_{p}")
                    nc.vector.tensor_copy(dn[:], au[64:65, :])
                    rc = pdn.tile([1, 2 * QC], F32, tag="rc", name=f"rc{j}_{p}")
                    nc.vector.reciprocal_approx_fast(rc[:], dn[:])
                    bcb = pbc.tile([64, 2 * QC], F32, tag="bc",
                                   name=f"bc{j}_{p}")
                    nc.gpsimd.partition_broadcast(bcb[:], rc[:])
                    for hi, hr in ((0, 0), (1, 64)):
                        nc.gpsimd.tensor_tensor(
                            aT_t[j][hr:hr + 64, q0:q0 + QC],
                            au[0:64, hi * QC:(hi + 1) * QC],
                            bcb[:, hi * QC:(hi + 1) * QC],
                            op=MULT,
                        )

            # ---------------- schedule ----------------
            # prologue: QK(0) + all V (PE warm-up while DMAs stream)
            for piece in qk_pieces(0):
                piece()
            for sb in range(NS):
                for piece in v_pieces(sb):
                    piece()

            # windows 0..2: attention(j) + QK(j+1) as fill.  Fill only the
            # first NP-1 passes so the next window's first scores never
            # wait on a bias-add landing at the window edge.
            for j in range(NJ - 1):
                qk = qk_pieces(j + 1)
                per = (len(qk) + NP - 2) // (NP - 1)
                fill = [qk[p * per:(p + 1) * per] for p in range(NP - 1)]
                fill.append([])
                attention_pair(j, fill)

            # window 3: attention(3) + out-proj of s-blocks, gated one FULL
            # pass after the pass that produced their aT columns so the
            # in-order PE never head-of-line blocks on the normalize chain
            # (pass p covers q-chunk p: sb 4(p-2)..4(p-1)-1 in pass p).
            fill = [[] for _ in range(NP)]
            for p in range(2, NP):
                for sb in range(4 * (p - 2), 4 * (p - 1)):
                    fill[p].extend(outproj_pieces(sb))
            attention_pair(NJ - 1, fill)

            # epilogue: remaining out-proj
            for sb in range(4 * (NP - 2), NS):
                for piece in outproj_pieces(sb):
                    piece()

    nc.compile()
    return nc


def _get_nc(dt_mode):
    if dt_mode not in _CACHE:
        _CACHE[dt_mode] = _build(dt_mode)
    return _CACHE[dt_mode]


def make_in_maps(x, Wq_w, Wq_b, Wk_w, Wk_b, Wv_w, Wv_b, Wo_w, Wo_b, np_dt):
    in_maps = []
    for core in range(N_CORES):
        b, half = core // 2, core % 2
        sl = slice(half * DH, (half + 1) * DH)
        in_maps.append({
            "xT": np.ascontiguousarray(x[b].T).astype(np_dt),
            "wq": np.ascontiguousarray(Wq_w[:, sl]).astype(np_dt),
            "wk": np.ascontiguousarray(Wk_w[:, sl]).astype(np_dt),
            "wv": np.ascontiguousarray(Wv_w[:, sl]).astype(np_dt),
            "wo": np.ascontiguousarray(Wo_w[sl, :]).astype(np_dt),
            "bq": np.ascontiguousarray(Wq_b[sl].reshape(-1, 128).T),
            "bk": np.ascontiguousarray(Wk_b[sl].reshape(-1, 128).T),
            "bv": np.broadcast_to(Wv_b[sl], (128, DH)).copy(),
        })
    return in_maps


def kernel(x, Wq_w, Wq_b, Wk_w, Wk_b, Wv_w, Wv_b, Wo_w, Wo_b):
    from concourse.bass_utils import run_bass_kernel_spmd

    np_dt = ml_dtypes.bfloat16 if DT_MODE == "bf16" else np.float32

    args = [np.asarray(a, np.float32) for a in
            (x, Wq_w, Wq_b, Wk_w, Wk_b, Wv_w, Wv_b, Wo_w, Wo_b)]
    x, Wq_w, Wq_b, Wk_w, Wk_b, Wv_w, Wv_b, Wo_w, Wo_b = args

    nc = _get_nc(DT_MODE)
    in_maps = make_in_maps(x, Wq_w, Wq_b, Wk_w, Wk_b, Wv_w, Wv_b, Wo_w, Wo_b,
                           np_dt)
    res = run_bass_kernel_spmd(nc, in_maps, list(range(N_CORES)))

    out = np.empty((B, S, D), np.float32)
    for b in range(B):
        out[b] = res.results[2 * b]["out"] + res.results[2 * b + 1]["out"] + Wo_b
    return out


# revision 28
# speedup vs baseline: 1.2700x; 1.1218x over previous
"""Trainium2 Bass kernel for nn_MultiHeadAttention (B=4, S=2048, D=1024, H=16).

Sharding: 8 cores = batch(4) x head-half(2).  Each core computes, for its
batch element, 8 of the 16 heads: QKV projections against column-sliced
weights, causal attention, and the output projection against the matching
row-slice of Wo.  The two partial outputs per batch element are summed on
the host (replaces the tensor-parallel all-reduce), and Wo_b is added there.

Attention runs in the transposed-scores layout scoresT[k, q]; the softmax
denominator comes free from an all-ones column appended to V (row 64 of the
PV psum accumulator).  Heads are processed in PAIRS sharing one [128, 1024]
scores psum tile (head A in cols 0:512, head B in 512:1024) so one ACTIVATE
exps both heads' scores; q is chunked at 512.

The whole kernel is software-pipelined around the ACT engine's exp stream
(the irreducible ~120us of work): Q/K projections for the NEXT head pair
and output-projection tiles for finished pairs are injected as fill between
attention steps so the PE never idles (idle gaps also drop the PE's DVFS
p-state from 2.4 to 1.2 GHz).  PSUM: 2 scores bufs (4 banks) + 1 shared PV
accumulator (2 banks) + 2 fill bufs (2 banks).
"""

import sys

if "/opt/trn_rl_repo" not in sys.path:
    sys.path.insert(0, "/opt/trn_rl_repo")

import numpy as np
import ml_dtypes

B, S, D = 4, 2048, 1024
H, HD = 16, 64
HH = H // 2          # heads per core
DH = D // 2          # local attention feature dim (HH * HD)
N_CORES = 8
QC = 512             # q-chunk per attention pass (1 psum bank per head)

# matmul dtype mode: "bf16" (fast, ~3e-3 rel err) | "f32" (exact, 4x PE cost)
DT_MODE = "bf16"

_CACHE = {}


def _build(dt_mode):
    import concourse.bass as bass
    import concourse.mybir as mybir
    from concourse import bacc
    from concourse.tile import TileContext
    from concourse.masks import make_upper_triangular

    F32 = mybir.dt.float32
    if dt_mode == "bf16":
        DT = mybir.dt.bfloat16
    elif dt_mode == "f32":
        DT = mybir.dt.float32
    else:
        raise ValueError(dt_mode)
    FP8 = mybir.dt.float8e4
    DR = mybir.MatmulPerfMode.DoubleRow

    ADD = mybir.AluOpType.add
    MULT = mybir.AluOpType.mult
    EXP = mybir.ActivationFunctionType.Exp

    nc = bacc.Bacc("TRN2", target_bir_lowering=False, debug=False,
                   num_devices=N_CORES)

    xT = nc.dram_tensor("xT", [D, S], DT, kind="ExternalInput").ap()
    wq = nc.dram_tensor("wq", [D, DH], DT, kind="ExternalInput").ap()
    wk = nc.dram_tensor("wk", [D, DH], DT, kind="ExternalInput").ap()
    wv = nc.dram_tensor("wv", [D, DH], DT, kind="ExternalInput").ap()
    wo = nc.dram_tensor("wo", [DH, D], DT, kind="ExternalInput").ap()
    bq = nc.dram_tensor("bq", [128, DH // 128], F32, kind="ExternalInput").ap()
    bk = nc.dram_tensor("bk", [128, DH // 128], F32, kind="ExternalInput").ap()
    bv = nc.dram_tensor("bv", [128, DH], F32, kind="ExternalInput").ap()
    out = nc.dram_tensor("out", [S, D], F32, kind="ExternalOutput").ap()

    ND = D // 128        # 8 contraction tiles over D
    NS = S // 128        # 16 s-blocks
    NJ = DH // 128       # 4 head-pair tiles of the local 512 dim
    NSC = S // 512       # 4 columns of 512 over S
    NP = S // QC         # 4 q-chunk passes

    with TileContext(nc) as tc:
        with (
            tc.tile_pool(name="persist", bufs=1) as pp,
            tc.tile_pool(name="xt", bufs=ND) as pxt,
            tc.tile_pool(name="wqk", bufs=2 * ND) as pwqk,
            tc.tile_pool(name="wv", bufs=ND) as pwv,
            tc.tile_pool(name="wo", bufs=NJ) as pwo,
            tc.tile_pool(name="qT", bufs=NJ) as pqT,
            tc.tile_pool(name="kT", bufs=NJ) as pkT,
            tc.tile_pool(name="vaug", bufs=NS) as pv,
            tc.tile_pool(name="attnT", bufs=NJ) as pattnT,
            tc.tile_pool(name="exp", bufs=3) as pexp,
            tc.tile_pool(name="au", bufs=2) as pau,
            tc.tile_pool(name="dn", bufs=4) as pdn,
            tc.tile_pool(name="bc", bufs=2) as pbc,
            tc.tile_pool(name="ostage", bufs=4) as post,
            tc.tile_pool(name="scps", bufs=2, space="PSUM") as pscps,
            tc.tile_pool(name="atps", bufs=1, space="PSUM") as patps,
            tc.tile_pool(name="auxps", bufs=2, space="PSUM") as pauxps,
        ):
            # ---- input DMAs (ordered by first use) ----
            wq_t, wk_t, xt_t = [], [], []
            for db in range(ND):
                tq = pwqk.tile([128, DH], DT, tag="wqk", name=f"wq{db}")
                nc.sync.dma_start(tq[:], wq[db * 128:(db + 1) * 128, :])
                wq_t.append(tq)
                tx = pxt.tile([128, S], DT, tag="xt", name=f"xt{db}")
                nc.sync.dma_start(tx[:], xT[db * 128:(db + 1) * 128, :])
                xt_t.append(tx)
            for db in range(ND):
                tk = pwqk.tile([128, DH], DT, tag="wqk", name=f"wk{db}")
                nc.sync.dma_start(tk[:], wk[db * 128:(db + 1) * 128, :])
                wk_t.append(tk)
            wv_t = []
            for db in range(ND):
                t = pwv.tile([128, DH], DT, tag="wv", name=f"wv{db}")
                nc.sync.dma_start(t[:], wv[db * 128:(db + 1) * 128, :])
                wv_t.append(t)
            bq_t = pp.tile([128, NJ], F32, tag="bq")
            nc.sync.dma_start(bq_t[:], bq[:])
            bk_t = pp.tile([128, NJ], F32, tag="bk")
            nc.sync.dma_start(bk_t[:], bk[:])
            bv_t = pp.tile([128, DH], F32, tag="bv")
            nc.sync.dma_start(bv_t[:], bv[:])
            wo_t = []
            for db in range(NJ):
                t = pwo.tile([128, D], DT, tag="wo", name=f"wo{db}")
                nc.sync.dma_start(t[:], wo[db * 128:(db + 1) * 128, :])
                wo_t.append(t)

            # ---- constants ----
            ones_t = pp.tile([128, HH], F32, tag="ones")
            nc.gpsimd.memset(ones_t[:], 1.0)
            ones1_t = pp.tile([1, 64], F32, tag="ones1")
            nc.gpsimd.memset(ones1_t[:], 1.0)
            # causal mask for diagonal 128x128 squares of scoresT[k, q]:
            # valid (k <= q) <=> partition p <= free f -> upper-tri incl
            # diag; two side-by-side copies (one per head of a pair).
            mask_f = pp.tile([128, 128], F32, tag="maskf")
            make_upper_triangular(nc, mask_f[:], val=1.0, diag=True)
            mask2 = pp.tile([128, 256], DT, tag="mask2")
            nc.vector.tensor_copy(mask2[:, 0:128], mask_f[:])
            nc.vector.tensor_copy(mask2[:, 128:256], mask_f[:])
            mask23 = mask2[:].rearrange("p (h c) -> p h c", h=2)

            # persistent activation buffers
            qT_t = [pqT.tile([128, S], DT, tag="qT", name=f"qT{i}")
                    for i in range(NJ)]
            kT_t = [pkT.tile([128, S], DT, tag="kT", name=f"kT{i}")
                    for i in range(NJ)]
            v_t = [pv.tile([128, HH * (HD + 1)], DT, tag="vaug",
                           name=f"vaug{i}") for i in range(NS)]
            aT_t = [pattnT.tile([128, S], DT, tag="attnT", name=f"attnT{i}")
                    for i in range(NJ)]

            # ---------- fill-work generators (2 matmuls per piece) ----------
            def qk_pieces(j):
                """Q/K projection for head pair j: chunks of ~0.4us pieces.
                A chunk = one psum accumulation group (must not be split
                around another aux-pool allocation)."""
                chunks = []
                for nm, w_t, bias_t, dstT in (
                    ("q", wq_t, bq_t, qT_t), ("k", wk_t, bk_t, kT_t)
                ):
                    for sc in range(NSC):
                        box = {}
                        pieces = []
                        for db0 in range(0, ND, 2):
                            def piece(db0=db0, nm=nm, w_t=w_t, bias_t=bias_t,
                                      dstT=dstT, sc=sc, j=j, box=box):
                                if db0 == 0:
                                    box["t"] = pauxps.tile(
                                        [128, 512], F32, tag="aux",
                                        name=f"qk{nm}{j}_{sc}")
                                for db in (db0, db0 + 1):
                                    nc.tensor.matmul(
                                        box["t"][:],
                                        lhsT=w_t[db][:, j * 128:(j + 1) * 128],
                                        rhs=xt_t[db][:, sc * 512:(sc + 1) * 512],
                                        start=(db == 0), stop=(db == ND - 1),
                                    )
                                if db0 == ND - 2:
                                    nc.vector.tensor_scalar_add(
                                        dstT[j][:, sc * 512:(sc + 1) * 512],
                                        box["t"][:], bias_t[:, j:j + 1],
                                    )
                            pieces.append(piece)
                        chunks.append(pieces)
                return chunks

            def v_pieces(sb):
                """V projection for s-block sb (one chunk of 4 pieces)."""
                pieces = []
                box = {}
                for db0 in range(0, ND, 2):
                    def piece(db0=db0, sb=sb, box=box):
                        if db0 == 0:
                            box["t"] = pauxps.tile([128, 512], F32, tag="aux",
                                                   name=f"vps{sb}")
                        for db in (db0, db0 + 1):
                            nc.tensor.matmul(
                                box["t"][:],
                                lhsT=xt_t[db][:, sb * 128:(sb + 1) * 128],
                                rhs=wv_t[db][:],
                                start=(db == 0), stop=(db == ND - 1),
                            )
                        if db0 == ND - 2:
                            vt = v_t[sb]
                            v3 = vt[:].rearrange("p (h e) -> p h e", e=HD + 1)
                            nc.vector.tensor_tensor(
                                v3[:, :, 0:HD],
                                box["t"][:].rearrange("p (h e) -> p h e", e=HD),
                                bv_t[:].rearrange("p (h e) -> p h e", e=HD),
                                op=ADD,
                            )
                            nc.vector.tensor_copy(
                                v3[:, :, HD:HD + 1],
                                ones_t[:].rearrange("p (h e) -> p h e", e=1),
                            )
                    pieces.append(piece)
                return [pieces]

            def outproj_pieces(sb):
                """Output projection for s-block sb: 2 chunks of 2 pieces."""
                chunks = []
                for jc in range(D // 512):
                    box = {}
                    pieces = []
                    for db0 in range(0, NJ, 2):
                        def piece(db0=db0, sb=sb, jc=jc, box=box):
                            if db0 == 0:
                                box["t"] = pauxps.tile(
                                    [128, 512], F32, tag="aux",
                                    name=f"ops{sb}_{jc}")
                            for db in (db0, db0 + 1):
                                nc.tensor.matmul(
                                    box["t"][:],
                                    lhsT=aT_t[db][:, sb * 128:(sb + 1) * 128],
                                    rhs=wo_t[db][:, jc * 512:(jc + 1) * 512],
                                    start=(db == 0), stop=(db == NJ - 1),
                                )
                            if db0 == NJ - 2:
                                ot = post.tile([128, 512], F32, tag="ostage",
                                               name=f"ot{sb}_{jc}")
                                nc.vector.tensor_copy(ot[:], box["t"][:])
                                nc.sync.dma_start(
                                    out[sb * 128:(sb + 1) * 128,
                                        jc * 512:(jc + 1) * 512],
                                    ot[:],
                                )
                        pieces.append(piece)
                    chunks.append(pieces)
                return chunks

            # ---------- attention for one head pair, with fill ----------
            # pending_norm holds the deferred tail of the previous pass's
            # softmax-normalize (rank-1 PE broadcast of 1/den + gpsimd
            # multiplies).  Deferring it into the NEXT pass's instruction
            # stream keeps the in-order PE from blocking on the DVE
            # reciprocal, and using a PE matmul instead of gpsimd's
            # partition_broadcast keeps gpsimd on a single library
            # (UNLOAD_LIB/LOAD_LIB thrash costs ~13us per pass otherwise).
            pending_norm = []

            def attention_pair(j, fill):
                """fill: list of per-pass CHUNK-lists (len NP)."""
                vcA = (2 * j) * (HD + 1)
                vcB = (2 * j + 1) * (HD + 1)
                for p in range(NP):
                    q0 = p * QC
                    nkb = (q0 + QC) // 128
                    at2 = patps.tile([65, 2 * QC], F32, tag="at",
                                     name=f"at{j}_{p}")
                    # flatten chunks; record the piece indices that are
                    # chunk boundaries (safe points for aux-psum reuse).
                    pfill = [pc for ch in fill[p] for pc in ch]
                    bounds = set()
                    n = 0
                    for ch in fill[p]:
                        bounds.add(n)
                        n += len(ch)
                    bounds.add(n)
                    fi = 0

                    def scores(kb):
                        k0 = kb * 128
                        lo = max(k0 - q0, 0)
                        sc2 = pscps.tile([128, 2 * QC], F32, tag="sc",
                                         name=f"sc{j}_{p}_{kb}")
                        for hi, hr in ((0, 0), (1, 64)):
                            nc.tensor.matmul(
                                sc2[:, hi * QC + lo:(hi + 1) * QC],
                                lhsT=kT_t[j][hr:hr + 64, k0:k0 + 128],
                                rhs=qT_t[j][hr:hr + 64, q0 + lo:q0 + QC],
                                start=True, stop=True,
                            )
                        return sc2

                    def exp_pv(kb, sc2):
                        k0 = kb * 128
                        lo = max(k0 - q0, 0)
                        et = pexp.tile([128, 2 * QC], DT, tag="exp",
                                       name=f"et{j}_{p}_{kb}")
                        et3 = et[:].rearrange("p (h c) -> p h c", h=2)
                        sc3 = sc2[:].rearrange("p (h c) -> p h c", h=2)
                        nc.scalar.activation(
                            et3[:, :, lo:QC], sc3[:, :, lo:QC],
                            EXP, scale=1.0 / np.sqrt(HD),
                        )
                        if k0 >= q0:
                            nc.vector.tensor_tensor(
                                et3[:, :, lo:lo + 128],
                                et3[:, :, lo:lo + 128],
                                mask23, op=MULT,
                            )
                        for hi, vc in ((0, vcA), (1, vcB)):
                            nc.tensor.matmul(
                                at2[0:65, hi * QC + lo:(hi + 1) * QC],
                                lhsT=v_t[kb][:, vc:vc + HD + 1],
                                rhs=et[:, hi * QC + lo:(hi + 1) * QC],
                                start=(kb == 0), stop=(kb == nkb - 1),
                            )

                    pend = {}
                    for kb in range(min(2, nkb)):
                        pend[kb] = scores(kb)
                    for kb in range(nkb):
                        want = ((kb + 1) * len(pfill)) // nkb
                        while fi < want:
                            pfill[fi]()
                            fi += 1
                        # exp_pv(kb) BEFORE scores(kb+2): the scps pool has
                        # 2 bufs, so scores(kb+2) reuses sc2(kb)'s buffer
                        # and its WAR dep must see exp(kb) already issued.
                        exp_pv(kb, pend.pop(kb))
                        if kb == 2 and pending_norm:
                            # finish any open fill chunk first: the deferred
                            # normalize allocates from the same aux psum
                            # pool and must not land inside an open
                            # accumulation group.
                            while fi not in bounds:
                                pfill[fi]()
                                fi += 1
                            pending_norm.pop(0)()
                        if kb + 2 < nkb:
                            pend[kb + 2] = scores(kb + 2)
                    while fi < len(pfill):
                        pfill[fi]()
                        fi += 1

                    # one DVE copy frees the at2 psum slot; reciprocal of
                    # the denominator row runs now (DVE only); broadcast +
                    # multiplies are deferred one pass (see pending_norm).
                    au = pau.tile([65, 2 * QC], F32, tag="au",
                                  name=f"au{j}_{p}")
                    nc.vector.tensor_copy(au[:], at2[0:65, :])
                    dn = pdn.tile([1, 2 * QC], F32, tag="dn", name=f"dn{j}_{p}")
                    nc.vector.tensor_copy(dn[:], au[64:65, :])
                    rc = pdn.tile([1, 2 * QC], F32, tag="rc", name=f"rc{j}_{p}")
                    nc.vector.reciprocal_approx_fast(rc[:], dn[:])

                    def norm_tail(j=j, q0=q0, au=au, rc=rc, tn=f"{j}_{p}"):
                        bcb = pbc.tile([64, 2 * QC], F32, tag="bc",
                                       name=f"bc{tn}")
                        for c in range(2):
                            bp = pauxps.tile([64, QC], F32, tag="aux",
                                             name=f"bcps{tn}_{c}")
                            nc.tensor.matmul(
                                bp[:], lhsT=ones1_t[0:1, :],
                                rhs=rc[0:1, c * QC:(c + 1) * QC],
                                start=True, stop=True,
                            )
                            nc.vector.tensor_copy(
                                bcb[:, c * QC:(c + 1) * QC], bp[:])
                        for hi, hr in ((0, 0), (1, 64)):
                            nc.gpsimd.tensor_tensor(
                                aT_t[j][hr:hr + 64, q0:q0 + QC],
                                au[0:64, hi * QC:(hi + 1) * QC],
                                bcb[:, hi * QC:(hi + 1) * QC],
                                op=MULT,
                            )
                    pending_norm.append(norm_tail)

            # ---------------- schedule ----------------
            # prologue: QK(0) + all V (PE warm-up while DMAs stream)
            for ch in qk_pieces(0):
                for piece in ch:
                    piece()
            for sb in range(NS):
                for ch in v_pieces(sb):
                    for piece in ch:
                        piece()

            # windows 0..2: attention(j) + QK(j+1) as fill.  Fill only the
            # first NP-1 passes so the next window's first scores never
            # wait on a bias-add landing at the window edge.
            for j in range(NJ - 1):
                qk = qk_pieces(j + 1)
                per = (len(qk) + NP - 2) // (NP - 1)
                fill = [qk[p * per:(p + 1) * per] for p in range(NP - 1)]
                fill.append([])
                attention_pair(j, fill)

            # window 3: attention(3) + out-proj of s-blocks, gated one FULL
            # pass after the pass that produced their aT columns so the
            # in-order PE never head-of-line blocks on the normalize chain
            # (pass p covers q-chunk p: sb 4(p-2)..4(p-1)-1 in pass p).
            fill = [[] for _ in range(NP)]
            for p in range(2, NP):
                for sb in range(4 * (p - 2), 4 * (p - 1)):
                    fill[p].extend(outproj_pieces(sb))
            attention_pair(NJ - 1, fill)

            # epilogue: out-proj for s-blocks 8..11 (their aT was written by
            # drained normalizes), then flush the final pass's normalize,
            # then the last s-blocks that depend on it.
            for sb in range(4 * (NP - 2), 4 * (NP - 1)):
                for ch in outproj_pieces(sb):
                    for piece in ch:
                        piece()
            while pending_norm:
                pending_norm.pop(0)()
            for sb in range(4 * (NP - 1), NS):
                for ch in outproj_pieces(sb):
                    for piece in ch:
                        piece()

    nc.compile()
    return nc


def _get_nc(dt_mode):
    if dt_mode not in _CACHE:
        _CACHE[dt_mode] = _build(dt_mode)
    return _CACHE[dt_mode]


def make_in_maps(x, Wq_w, Wq_b, Wk_w, Wk_b, Wv_w, Wv_b, Wo_w, Wo_b, np_dt):
    in_maps = []
    for core in range(N_CORES):
        b, half = core // 2, core % 2
        sl = slice(half * DH, (half + 1) * DH)
        in_maps.append({
            "xT": np.ascontiguousarray(x[b].T).astype(np_dt),
            "wq": np.ascontiguousarray(Wq_w[:, sl]).astype(np_dt),
            "wk": np.ascontiguousarray(Wk_w[:, sl]).astype(np_dt),
            "wv": np.ascontiguousarray(Wv_w[:, sl]).astype(np_dt),
            "wo": np.ascontiguousarray(Wo_w[sl, :]).astype(np_dt),
            "bq": np.ascontiguousarray(Wq_b[sl].reshape(-1, 128).T),
            "bk": np.ascontiguousarray(Wk_b[sl].reshape(-1, 128).T),
            "bv": np.broadcast_to(Wv_b[sl], (128, DH)).copy(),
        })
    return in_maps


def kernel(x, Wq_w, Wq_b, Wk_w, Wk_b, Wv_w, Wv_b, Wo_w, Wo_b):
    from concourse.bass_utils import run_bass_kernel_spmd

    np_dt = ml_dtypes.bfloat16 if DT_MODE == "bf16" else np.float32

    args = [np.asarray(a, np.float32) for a in
            (x, Wq_w, Wq_b, Wk_w, Wk_b, Wv_w, Wv_b, Wo_w, Wo_b)]
    x, Wq_w, Wq_b, Wk_w, Wk_b, Wv_w, Wv_b, Wo_w, Wo_b = args

    nc = _get_nc(DT_MODE)
    in_maps = make_in_maps(x, Wq_w, Wq_b, Wk_w, Wk_b, Wv_w, Wv_b, Wo_w, Wo_b,
                           np_dt)
    res = run_bass_kernel_spmd(nc, in_maps, list(range(N_CORES)))

    out = np.empty((B, S, D), np.float32)
    for b in range(B):
        out[b] = res.results[2 * b]["out"] + res.results[2 * b + 1]["out"] + Wo_b
    return out
